# revision 15
# baseline (speedup 1.0000x reference)
"""Trainium2 Bass kernel for nn_BuzCusStructureSim (dense_transformer).

Exact math simplifications (hold for ANY input values, no distribution
assumptions):
 - softmax over a trailing size-1 axis is exactly 1.0, so the _weighted_sum
   calls are plain sums over the trailing feature axis, and the final W_f
   mixing reduces to out = BS_out + CS_out.  W_bs/W_cs/W_f never affect the
   output.
 - the attention softmax (over the query axis s) is immediately contracted
   with Bt[s]:  BR[t,h] = (sum_s Bt[s] e[s,t]) / (sum_s e[s,t]) with
   e = exp(scores/16).  Both sums come out of one PE matmul with
   lhsT = [Bt | ones] (M=2) contracting s.
 - Q K^T = E (W1 W2^T) E^T: M_h = W1_h W2_h^T is precomputed once ([D,D],
   shared by all batches and both E tensors), halving projection matmuls.

Sharding: data-parallel over batch (16 batches / 8 cores = 2 per core).
Matmul dtype: float32r (TF32-like, ~1.6e-4 rel err, full PE rate at N>=256).
"""

import numpy as np

import concourse.bacc as bacc
import concourse.tile as tile
from concourse import mybir
from concourse.bass_utils import run_bass_kernel_spmd

B, S, D, H, FB, K, FC = 16, 512, 256, 8, 128, 64, 32
NCORES = 8
BL = B // NCORES  # batches per core
NT = S // 128     # 4 s-tiles / t-tiles
ND = D // 128     # 2 d-tiles

F32 = mybir.dt.float32
F32R = mybir.dt.float32r
AX = mybir.AxisListType
ALU = mybir.AluOpType
ACT = mybir.ActivationFunctionType
SCALE = 1.0 / 16.0  # 1/sqrt(D)


def _layernorm(nc, sm, trps, x, gamma_t, beta_t, out, ones_col, ones_row, eps):
    """LN over all 512 values of x [128(t_lo), 4(tt)] -> out [128, 4].

    out = (x - mean) * rsqrt(var + eps) * gamma_t + beta_t
    (negate gamma_t at load time if a global sign flip is needed).
    rsqrt via exp(-0.5*ln(.)) to stay inside the natural_log_exp table set.
    """
    lnin = sm.tile([128, 8], F32, tag="lnin")
    nc.vector.tensor_copy(lnin[:, 0:4], x)
    nc.vector.tensor_mul(lnin[:, 4:8], x, x)
    ps_s = trps.tile([1, 8], F32, tag="tr")
    nc.tensor.matmul(ps_s[:], ones_col[:], lnin[:])  # column sums over 128 parts
    ss = sm.tile([1, 8], F32, tag="ss")
    nc.vector.tensor_copy(ss[:], ps_s[:])
    st4 = sm.tile([1, 4], F32, tag="st4")  # [sum, sumsq, mean, ex2] scratch
    nc.vector.reduce_sum(st4[0:1, 0:1], ss[0:1, 0:4], axis=AX.X)
    nc.vector.reduce_sum(st4[0:1, 1:2], ss[0:1, 4:8], axis=AX.X)
    mr = sm.tile([1, 2], F32, tag="mr")  # [mean, rstd]
    nc.vector.tensor_scalar_mul(mr[0:1, 0:1], st4[0:1, 0:1], 1.0 / S)
    nc.vector.tensor_scalar_mul(st4[0:1, 2:3], st4[0:1, 1:2], 1.0 / S)  # E[x^2]
    nc.vector.tensor_mul(st4[0:1, 3:4], mr[0:1, 0:1], mr[0:1, 0:1])    # mean^2
    nc.vector.tensor_sub(st4[0:1, 0:1], st4[0:1, 2:3], st4[0:1, 3:4])  # var
    nc.scalar.activation(st4[0:1, 1:2], st4[0:1, 0:1], ACT.Ln, bias=eps)  # eps: AP [1,1]
    nc.scalar.activation(mr[0:1, 1:2], st4[0:1, 1:2], ACT.Exp, scale=-0.5)
    bc_ps = trps.tile([128, 2], F32, tag="tr")
    nc.tensor.matmul(bc_ps[:], ones_row[:], mr[:])  # broadcast [mean,rstd]
    bc = sm.tile([128, 2], F32, tag="bc")
    nc.vector.tensor_copy(bc[:], bc_ps[:])
    xm = sm.tile([128, 4], F32, tag="xm")
    nc.vector.tensor_scalar_sub(xm[:], x, bc[:, 0:1])
    nc.vector.tensor_scalar_mul(xm[:], xm[:], bc[:, 1:2])
    nc.vector.tensor_mul(xm[:], xm[:], gamma_t)
    nc.vector.tensor_add(out, xm[:], beta_t)


def build():
    nc = bacc.Bacc("TRN2")

    # ---- per-core I/O (batch-sharded on dim 0) ----
    b_t = nc.dram_tensor("b_target", [BL, S, FB], F32, kind="ExternalInput")
    b_i = nc.dram_tensor("b_infected", [BL, S, FB], F32, kind="ExternalInput")
    e_t = nc.dram_tensor("e_target", [BL, S, D], F32, kind="ExternalInput")
    e_i = nc.dram_tensor("e_infected", [BL, S, D], F32, kind="ExternalInput")
    c_t = nc.dram_tensor("c_target", [BL, S, K, FC], F32, kind="ExternalInput")
    c_i = nc.dram_tensor("c_infected", [BL, S, K, FC], F32, kind="ExternalInput")
    w1 = nc.dram_tensor("w1", [H, D, D], F32, kind="ExternalInput")
    w2 = nc.dram_tensor("w2", [H, D, D], F32, kind="ExternalInput")
    gbs = nc.dram_tensor("gamma_bs", [S], F32, kind="ExternalInput")
    bbs = nc.dram_tensor("beta_bs", [S], F32, kind="ExternalInput")
    gcs = nc.dram_tensor("gamma_cs", [S], F32, kind="ExternalInput")
    bcs = nc.dram_tensor("beta_cs", [S], F32, kind="ExternalInput")
    o_out = nc.dram_tensor("o_out", [BL, S], F32, kind="ExternalOutput")
    o_bs = nc.dram_tensor("o_bs", [BL, S], F32, kind="ExternalOutput")
    o_cs = nc.dram_tensor("o_cs", [BL, S], F32, kind="ExternalOutput")

    ident_d = nc.inline_tensor(np.eye(128, dtype=np.float32), name="ident")

    with tile.TileContext(nc) as tc:
        _emit(nc, tc, locals())
    nc.compile()
    return nc


def _emit(nc, tc, t):
    from contextlib import ExitStack

    with ExitStack() as ctx:
        const = ctx.enter_context(tc.tile_pool(name="const", bufs=1))
        wm = ctx.enter_context(tc.tile_pool(name="wm", bufs=1))

        ident = const.tile([128, 128], F32)
        nc.sync.dma_start(ident[:], t["ident_d"][:])
        ones_col = const.tile([128, 1], F32)
        nc.vector.memset(ones_col[:], 1.0)
        ones_row = const.tile([1, 128], F32)
        nc.vector.memset(ones_row[:], 1.0)
        eps_t = const.tile([1, 1], F32)
        nc.vector.memset(eps_t[:], 1e-16)

        # gamma/beta as [128(lo), 4(hi)] tiles; s = hi*128 + lo
        def ln_vec(name, neg):
            tl = const.tile([128, NT], F32, tag=name)
            ap = t[name].ap().rearrange("(hi lo) -> lo hi", lo=128)
            nc.sync.dma_start(tl[:], ap)
            if neg:
                nc.vector.tensor_scalar_mul(tl[:], tl[:], -1.0)
            return tl

        gbs_t = ln_vec("gbs", True)   # negated: folds the cosine minus sign
        bbs_t = ln_vec("bbs", False)
        gcs_t = ln_vec("gcs", False)
        bcs_t = ln_vec("bcs", False)

        # ---------- phase 0: M_h = W1_h @ W2_h^T  (f32r, [d, d'] tiles) ----
        m_sb = [wm.tile([128, ND, D], F32R, tag=f"m{h}", name=f"m{h}")
                for h in range(H)]
        with tc.tile_pool(name="wtmp", bufs=3) as wpool, \
             tc.tile_pool(name="wtr", bufs=2) as wtr, \
             tc.tile_pool(name="ps0", bufs=4, space="PSUM") as ps0:
            for h in range(H):
                wts = []
                for wi, w_dram in enumerate((t["w1"], t["w2"])):
                    wt = wtr.tile([128, ND, D], F32R, tag=f"wt{wi}")  # [e,(ee),d]
                    for dd in range(ND):
                        wraw = wpool.tile([128, D], F32, tag="wraw")
                        nc.sync.dma_start(
                            wraw[:], w_dram[h, dd * 128:(dd + 1) * 128, :])
                        for ee in range(ND):
                            ptr = ps0.tile([128, 128], F32, tag="p0")
                            nc.tensor.transpose(
                                ptr[:], wraw[:, ee * 128:(ee + 1) * 128],
                                ident[:])
                            nc.vector.tensor_copy(
                                wt[:, ee, dd * 128:(dd + 1) * 128], ptr[:])
                    wts.append(wt)
                w1tr, w2tr = wts
                for dt_ in range(ND):
                    psm = ps0.tile([128, D], F32, tag="pm")
                    for ee in range(ND):
                        nc.tensor.matmul(
                            psm[:],
                            w1tr[:, ee, dt_ * 128:(dt_ + 1) * 128],
                            w2tr[:, ee, :],
                            start=(ee == 0), stop=(ee == ND - 1))
                    nc.vector.tensor_copy(m_sb[h][:, dt_, :], psm[:])

        # ---------- steady-state pools ----------
        trps = ctx.enter_context(tc.tile_pool(name="trps", bufs=1, space="PSUM"))
        gps_p = ctx.enter_context(tc.tile_pool(name="gps", bufs=1, space="PSUM"))
        scps_p = ctx.enter_context(tc.tile_pool(name="scps", bufs=2, space="PSUM"))
        ndps_p = ctx.enter_context(tc.tile_pool(name="ndps", bufs=1, space="PSUM"))
        epool = ctx.enter_context(tc.tile_pool(name="epool", bufs=3))
        etp = ctx.enter_context(tc.tile_pool(name="etp", bufs=2))
        btp = ctx.enter_context(tc.tile_pool(name="btp", bufs=2))
        gsb_p = ctx.enter_context(tc.tile_pool(name="gsb", bufs=2))
        e1_p = ctx.enter_context(tc.tile_pool(name="e1", bufs=2))
        cp = ctx.enter_context(tc.tile_pool(name="cp", bufs=3))
        ctp = ctx.enter_context(tc.tile_pool(name="ctp", bufs=4))
        kp = ctx.enter_context(tc.tile_pool(name="kp", bufs=2))
        sm = ctx.enter_context(tc.tile_pool(name="sm", bufs=2))

        for b in range(BL):
            nd_sb = sm.tile([2 * H * 2, S], F32, tag="ndsb")
            ct_tiles = []
            for ti, (e_dram, b_dram, c_dram) in enumerate(
                    [(t["e_t"], t["b_t"], t["c_t"]),
                     (t["e_i"], t["b_i"], t["c_i"])]):
                # --- E^T [d, s] (f32r) via PE transpose ---
                et = etp.tile([128, ND, S], F32R, tag="et")
                for st in range(NT):
                    eraw = epool.tile([128, D], F32, tag="eraw")
                    nc.sync.dma_start(
                        eraw[:], e_dram[b, st * 128:(st + 1) * 128, :])
                    for dd in range(ND):
                        ptr = trps.tile([128, 128], F32, tag="tr")
                        nc.tensor.transpose(
                            ptr[:], eraw[:, dd * 128:(dd + 1) * 128], ident[:])
                        nc.vector.tensor_copy(
                            et[:, dd, st * 128:(st + 1) * 128], ptr[:])

                # --- Bt = sum_fb B  -> lhsT [s, (st), {Bt, 1}] (f32r) ---
                bto = btp.tile([128, NT, 2], F32R, tag="bto")
                btf = btp.tile([128, NT, 2], F32, tag="btf")
                nc.vector.memset(btf[:, :, 1:2], 1.0)
                for st in range(NT):
                    braw = epool.tile([128, FB], F32, tag="braw")
                    nc.sync.dma_start(
                        braw[:], b_dram[b, st * 128:(st + 1) * 128, :])
                    nc.vector.reduce_sum(
                        btf[:, st, 0:1], braw[:], axis=AX.X)
                nc.vector.tensor_copy(bto[:], btf[:])

                # --- C branch loads + inner-sum (overlaps with heads) ---
                ct_sb = ctp.tile([128, NT, K], F32, tag=f"ct{ti}")
                for st in range(NT):
                    c_sb = cp.tile([128, K, FC], F32, tag="c")
                    nc.sync.dma_start(
                        c_sb[:], c_dram[b, st * 128:(st + 1) * 128, :, :])
                    nc.vector.reduce_sum(ct_sb[:, st, :], c_sb[:], axis=AX.X)
                ct_tiles.append(ct_sb)

                # --- heads ---
                for h in range(H):
                    hh = ti * H + h
                    # G[d', s] = sum_d M[d, d'] ET[d, s]
                    gps = gps_p.tile([128, ND, S], F32, tag="g")
                    for dtp in range(ND):
                        for dd in range(ND):
                            nc.tensor.matmul(
                                gps[:, dtp, :],
                                m_sb[h][:, dd, dtp * 128:(dtp + 1) * 128],
                                et[:, dd, :],
                                start=(dd == 0), stop=(dd == ND - 1))
                    gsb = gsb_p.tile([128, ND, S], F32R, tag="gsb")
                    nc.vector.tensor_copy(gsb[:], gps[:])
                    # scores[s, t] = sum_d' G[d', s] ET[d', t]; exp via ACT
                    e1 = e1_p.tile([128, NT, S], F32R, tag="e1")
                    for st in range(NT):
                        scps = scps_p.tile([128, S], F32, tag="sc", bufs=3)
                        for dtp in range(ND):
                            nc.tensor.matmul(
                                scps[:],
                                gsb[:, dtp, st * 128:(st + 1) * 128],
                                et[:, dtp, :],
                                start=(dtp == 0), stop=(dtp == ND - 1))
                        nc.scalar.activation(
                            e1[:, st, :], scps[:], ACT.Exp, scale=SCALE)
                    # [num; den] rows via lhsT = [Bt | 1]
                    ndps = ndps_p.tile([2, S], F32, tag="nd", bufs=2)
                    for st in range(NT):
                        nc.tensor.matmul(
                            ndps[:],
                            bto[:, st, :],
                            e1[:, st, :],
                            start=(st == 0), stop=(st == NT - 1))
                    nds = sm.tile([2, S], F32, tag="nds")
                    nc.vector.tensor_copy(nds[:], ndps[:])
                    nc.sync.dma_start(nd_sb[2 * hh:2 * hh + 2, :], nds[:])

            # ---------- tail for batch b ----------
            ndtr = trps.tile([128, 128], F32, tag="tr")
            for tt in range(NT):
                nc.tensor.transpose(
                    ndtr[:, tt * 32:(tt + 1) * 32],
                    nd_sb[:, tt * 128:(tt + 1) * 128],
                    ident[0:32, 0:32])
            brnd = sm.tile([128, 128], F32, tag="brnd")
            nc.vector.tensor_copy(brnd[:], ndtr[:])
            brv = brnd[:].rearrange("p (a h2 two) -> p a h2 two",
                                    a=NT, h2=2 * H, two=2)
            rec = sm.tile([128, NT, 2 * H], F32, tag="rec")
            nc.vector.reciprocal(rec[:], brv[:, :, :, 1])
            br = sm.tile([128, NT, 2 * H], F32, tag="br")
            nc.vector.tensor_mul(br[:], brv[:, :, :, 0], rec[:])
            # cosine over h (8): a = target cols 0..7, c2 = infected cols 8..15
            a = br[:, :, 0:H]
            c2 = br[:, :, H:2 * H]
            pr = sm.tile([128, NT, H], F32, tag="pr")
            red = sm.tile([128, NT, 3], F32, tag="red")  # dot, na, nb
            nc.vector.tensor_mul(pr[:], a, c2)
            nc.vector.reduce_sum(red[:, :, 0], pr[:], axis=AX.X)
            nc.vector.tensor_mul(pr[:], a, a)
            nc.vector.reduce_sum(red[:, :, 1], pr[:], axis=AX.X)
            nc.vector.tensor_mul(pr[:], c2, c2)
            nc.vector.reduce_sum(red[:, :, 2], pr[:], axis=AX.X)
            nc.vector.tensor_scalar_max(red[:, :, 1:3], red[:, :, 1:3], 1e-12)
            m4 = sm.tile([128, NT], F32, tag="m4")
            nc.vector.tensor_mul(m4[:], red[:, :, 1], red[:, :, 2])
            nc.scalar.activation(m4[:], m4[:], ACT.Ln)
            nc.scalar.activation(m4[:], m4[:], ACT.Exp, scale=-0.5)
            cosn = sm.tile([128, NT], F32, tag="cosn")  # = -cos_ref
            nc.vector.tensor_mul(cosn[:], red[:, :, 0], m4[:])
            bs_o = sm.tile([128, NT], F32, tag="bs_o")
            _layernorm(nc, sm, trps, cosn[:], gbs_t[:], bbs_t[:], bs_o[:],
                       ones_col, ones_row, eps_t[0:1, :])

            # --- C branch tail: Cv then LN ---
            ct_sb, ci_sb = ct_tiles
            cm = kp.tile([128, NT, K], F32, tag="cm")
            nc.vector.tensor_add(cm[:], ct_sb[:], ci_sb[:])
            nc.vector.tensor_scalar_mul(cm[:], cm[:], 0.5)
            for x in (ct_sb, ci_sb, cm):  # clip in place
                nc.vector.tensor_scalar(
                    x[:], x[:], 1e-7, 1.0, op0=ALU.max, op1=ALU.min)
            lt = kp.tile([128, NT, K], F32, tag="lt")
            li = kp.tile([128, NT, K], F32, tag="li")
            lm = kp.tile([128, NT, K], F32, tag="lm")
            nc.scalar.activation(lt[:], ct_sb[:], ACT.Ln)
            nc.scalar.activation(li[:], ci_sb[:], ACT.Ln)
            nc.scalar.activation(lm[:], cm[:], ACT.Ln)
            nc.vector.tensor_sub(lt[:], lt[:], lm[:])
            nc.vector.tensor_mul(lt[:], lt[:], ct_sb[:])
            nc.vector.tensor_sub(li[:], li[:], lm[:])
            nc.vector.tensor_mul(li[:], li[:], ci_sb[:])
            nc.vector.tensor_add(lt[:], lt[:], li[:])
            cv = sm.tile([128, NT], F32, tag="cv")
            nc.vector.reduce_sum(cv[:], lt[:], axis=AX.X)
            nc.vector.tensor_scalar_mul(cv[:], cv[:], 0.5)
            cs_o = sm.tile([128, NT], F32, tag="cs_o")
            _layernorm(nc, sm, trps, cv[:], gcs_t[:], bcs_t[:], cs_o[:],
                       ones_col, ones_row, eps_t[0:1, :])

            fin = sm.tile([128, NT], F32, tag="fin")
            nc.vector.tensor_add(fin[:], bs_o[:], cs_o[:])
            for dram, tl in ((t["o_out"], fin), (t["o_bs"], bs_o),
                             (t["o_cs"], cs_o)):
                for tt in range(NT):
                    nc.sync.dma_start(
                        dram[b, tt * 128:(tt + 1) * 128], tl[:, tt])


_NC_CACHE = []
TRACE = False
LAST_RESULT = []


def kernel(**inputs):
    if not _NC_CACHE:
        _NC_CACHE.append(build())
    nc = _NC_CACHE[0]

    def shard(x, i):
        return np.ascontiguousarray(x[i * BL:(i + 1) * BL])

    in_maps = []
    for i in range(NCORES):
        in_maps.append({
            "b_target": shard(inputs["B_target"], i),
            "b_infected": shard(inputs["B_infected"], i),
            "e_target": shard(inputs["E_target"], i),
            "e_infected": shard(inputs["E_infected"], i),
            "c_target": shard(inputs["C_target"], i),
            "c_infected": shard(inputs["C_infected"], i),
            "w1": np.ascontiguousarray(inputs["W1"]),
            "w2": np.ascontiguousarray(inputs["W2"]),
            "gamma_bs": np.ascontiguousarray(inputs["gamma_bs"]),
            "beta_bs": np.ascontiguousarray(inputs["beta_bs"]),
            "gamma_cs": np.ascontiguousarray(inputs["gamma_cs"]),
            "beta_cs": np.ascontiguousarray(inputs["beta_cs"]),
        })
    res = run_bass_kernel_spmd(nc, in_maps, list(range(NCORES)), trace=TRACE)
    LAST_RESULT.clear()
    LAST_RESULT.append(res)
    out = np.concatenate([r["o_out"] for r in res.results], axis=0)
    bs = np.concatenate([r["o_bs"] for r in res.results], axis=0)
    cs = np.concatenate([r["o_cs"] for r in res.results], axis=0)
    return (out, bs, cs)


def bench(iters=32, **inputs):
    """Amortized real-HW timing: pipelined repeated NEFF executions with
    inputs resident on device. Returns (per_iter_seconds, results_list)."""
    import time
    import jax
    from jax.sharding import Mesh, PartitionSpec, NamedSharding
    from jax.experimental.shard_map import shard_map
    from concourse import bass2jax
    from concourse import mybir as _mb

    if not _NC_CACHE:
        _NC_CACHE.append(build())
    nc = _NC_CACHE[0]
    bass2jax.install_neuronx_cc_hook()

    def shard(x, i):
        return np.ascontiguousarray(x[i * BL:(i + 1) * BL])

    key_map = {
        "b_target": "B_target", "b_infected": "B_infected",
        "e_target": "E_target", "e_infected": "E_infected",
        "c_target": "C_target", "c_infected": "C_infected",
        "w1": "W1", "w2": "W2", "gamma_bs": "gamma_bs",
        "beta_bs": "beta_bs", "gamma_cs": "gamma_cs", "beta_cs": "beta_cs",
    }
    partition_name = (nc.partition_id_tensor.name
                      if nc.partition_id_tensor else None)
    in_names, out_names, out_avals, zero_outs = [], [], [], []
    for alloc in nc.m.functions[0].allocations:
        if not isinstance(alloc, _mb.MemoryLocationSet):
            continue
        name = alloc.memorylocations[0].name
        if alloc.kind == "ExternalInput" and name != partition_name:
            in_names.append(name)
        elif alloc.kind == "ExternalOutput":
            out_names.append(name)
            shp, dt = tuple(alloc.tensor_shape), _mb.dt.np(alloc.dtype)
            out_avals.append(jax.core.ShapedArray(shp, dt))
            zero_outs.append(np.zeros(shp, dt))
    n_params = len(in_names)
    all_names = in_names + out_names
    if partition_name is not None:
        all_names.append(partition_name)

    def _body(*args):
        operands = list(args)
        if partition_name is not None:
            operands.append(bass2jax.partition_id_tensor())
        return tuple(_bass_exec(operands))

    def _bass_exec(operands):
        return bass2jax._bass_exec_p.bind(
            *operands,
            out_avals=tuple(out_avals),
            in_names=tuple(all_names),
            out_names=tuple(out_names),
            lowering_input_output_aliases=(),
            sim_require_finite=True,
            sim_require_nnan=True,
            nc=nc,
        )

    devices = jax.devices()[:NCORES]
    mesh = Mesh(np.asarray(devices), ("core",))
    n_outs = len(out_names)
    donate = tuple(range(n_params, n_params + n_outs))
    sharded = jax.jit(
        shard_map(_body, mesh=mesh,
                  in_specs=(PartitionSpec("core"),) * (n_params + n_outs),
                  out_specs=(PartitionSpec("core"),) * n_outs,
                  check_rep=False),
        donate_argnums=donate, keep_unused=True)

    concat_in = []
    for n in in_names:
        full = np.asarray(inputs[key_map[n]], np.float32)
        if key_map[n] in ("B_target", "B_infected", "E_target", "E_infected",
                          "C_target", "C_infected"):
            concat_in.append(np.ascontiguousarray(full))  # already (16, ...)
        else:
            concat_in.append(np.concatenate([full] * NCORES, axis=0))
    sh = NamedSharding(mesh, PartitionSpec("core"))
    concat_in_dev = [jax.device_put(x, sh) for x in concat_in]
    concat_zeros = [np.zeros((NCORES * z.shape[0], *z.shape[1:]), z.dtype)
                    for z in zero_outs]

    # warmup (compile + 2 runs)
    outs = sharded(*concat_in_dev, *[np.copy(z) for z in concat_zeros])
    jax.block_until_ready(outs)
    outs = sharded(*concat_in_dev, *[np.copy(z) for z in concat_zeros])
    jax.block_until_ready(outs)
    t0 = time.perf_counter()
    last = None
    for _ in range(iters):
        last = sharded(*concat_in_dev, *[np.copy(z) for z in concat_zeros])
    jax.block_until_ready(last)
    t1 = time.perf_counter()
    return (t1 - t0) / iters, [np.asarray(o) for o in last]
